# revision 23
# baseline (speedup 1.0000x reference)
"""Causal multi-head attention (RoPE) forward for Trainium2, sharded over 8 NeuronCores.

Problem (hardcoded): B=2, S=2048, E=128, H=16, D=128, inner=2048.
  out = softmax(causal(rope(q@Wq) @ rope(q@Wk).T / sqrt(D))) @ (q@Wv) @ Wo

Sharding: tensor-parallel over heads - core c owns heads {2c, 2c+1} for both
batches (4 attention units/core). Each core computes its heads' projections,
attention, and partial W_o output (row-shard); host sums the 8 partials.

v2 design notes (vs the f32r baseline, 195 us):
 - All matmul operands fp16 (1 cyc/row at any p-state/width; halves weight
   loads + SBUF). Accuracy budget: gate is 2e-2, fp16 lands ~1e-3.
 - Window-interleaved emission: AV blocks run one slot behind their score
   blocks so the PE always has independent matmuls while ACT drains exps
   (keeps the PE p-state ramped at 2.4 GHz).
 - PSUM slots are [128,1024] spanning 2 banks: score-chunk PAIRS share a
   slot so full-width exps are 1024 wide (halves ACT per-instr overhead);
   proj W/W' pairs share a slot so each rope mul is one 1024-wide DVE op.
 - o_h transposed via XBAR DMA (dma_start_transpose) instead of PE
   transpose + DVE evict.
 - W_o partials for both heads accumulate into ONE PSUM tile; evicted by
   direct PSUM->DRAM DMA if supported (else DVE copy).
 - Engine split: DVE = rope muls, V-evict, normalize, recip; Pool(gpsimd,
   SBUF-only!) = rope adds, diag tril mask; ACT = exp only.
 - Denominator via the 129th ones-column of [V | 1] (moving operand).
"""

import os
import sys
import numpy as np

for _p in ("/root/.axon_site", "/root/.axon_site/_ro/trn_rl_repo",
           "/root/.axon_site/_ro/pypackages", "/opt/trn_rl_repo"):
    if os.path.isdir(_p) and _p not in sys.path:
        sys.path.append(_p)

from collections import deque
from contextlib import ExitStack

import concourse.bacc as bacc
import concourse.mybir as mybir
import concourse.tile as tile
from concourse import bass_utils

F32 = mybir.dt.float32
F16 = mybir.dt.float16
AF = mybir.ActivationFunctionType

B, S, E = 2, 2048, 128
H, D = 16, 128
NCORES = 8
HPC = H // NCORES          # heads per core = 2
WIN = 512                  # token window
NW = S // WIN              # windows per batch = 4
SCALE = 1.0 / np.sqrt(D)

PSUM_DMA_OUT = False       # DMA cannot read PSUM (bass asserts SBUF/DRAM src)

_CACHE = {}


def _build():
    nc = bacc.Bacc("TRN2", target_bir_lowering=False, debug=False)

    qT_d = nc.dram_tensor("qT", [E, B * S], F16, kind="ExternalInput").ap()
    wqk_d = nc.dram_tensor("wqk", [E, 8 * D], F16, kind="ExternalInput").ap()
    wv_d = nc.dram_tensor("wv", [E, HPC * D], F16, kind="ExternalInput").ap()
    wo_d = nc.dram_tensor("wo", [D, HPC * E], F16, kind="ExternalInput").ap()
    cs_d = nc.dram_tensor("csT", [D, 2, S], F16, kind="ExternalInput").ap()
    tril_d = nc.dram_tensor("tril", [128, 128], F16, kind="ExternalInput").ap()
    id_d = nc.dram_tensor("ident", [128, 128], F16, kind="ExternalInput").ap()
    outp_d = nc.dram_tensor("outp", [B * E, S], F32, kind="ExternalOutput").ap()

    with tile.TileContext(nc) as tc, ExitStack() as ctx:
        const = ctx.enter_context(tc.tile_pool(name="const", bufs=1))
        qkp = ctx.enter_context(tc.tile_pool(name="qkp", bufs=1))
        vhp = ctx.enter_context(tc.tile_pool(name="vhp", bufs=1))
        tmp = ctx.enter_context(tc.tile_pool(name="tmp", bufs=3))
        expp = ctx.enter_context(tc.tile_pool(name="expp", bufs=44))
        outp = ctx.enter_context(tc.tile_pool(name="outp", bufs=3))
        psS = ctx.enter_context(tc.tile_pool(name="psS", bufs=2, space="PSUM"))
        psT = ctx.enter_context(tc.tile_pool(name="psT", bufs=1, space="PSUM"))
        psP = ctx.enter_context(tc.tile_pool(name="psP", bufs=2, space="PSUM"))
        psA = ctx.enter_context(tc.tile_pool(name="psA", bufs=2, space="PSUM"))
        psF = ctx.enter_context(tc.tile_pool(name="psF", bufs=1, space="PSUM"))

        # ---- constant loads, ordered so slot-0's deps land first ----
        wqk_t = const.tile([128, 8 * D], F16, tag="wqk")
        nc.sync.dma_start(wqk_t[:], wqk_d[:])
        qt_w = [None] * (B * NW)
        cs_w = [None] * NW
        for w in range(NW):
            t = const.tile([128, WIN], F16, tag=f"qt{w}", name=f"qt{w}")
            nc.sync.dma_start(t[:], qT_d[:, w * WIN:(w + 1) * WIN])
            qt_w[w] = t
            c = const.tile([128, 2, WIN], F16, tag=f"cs{w}", name=f"cs{w}")
            nc.sync.dma_start(c[:], cs_d[:, :, w * WIN:(w + 1) * WIN])
            cs_w[w] = c
        wv_t = const.tile([128, HPC * D], F16, tag="wv")
        nc.sync.dma_start(wv_t[:], wv_d[:])
        for i in range(NW, B * NW):
            t = const.tile([128, WIN], F16, tag=f"qt{i}", name=f"qt{i}")
            nc.sync.dma_start(t[:], qT_d[:, i * WIN:(i + 1) * WIN])
            qt_w[i] = t
        wo_t = const.tile([128, HPC * E], F16, tag="wo")
        nc.sync.dma_start(wo_t[:], wo_d[:])
        tril_t = const.tile([128, 128], F16, tag="tril")
        nc.sync.dma_start(tril_t[:], tril_d[:])
        id_t = const.tile([128, 128], F16, tag="ident")
        nc.sync.dma_start(id_t[:], id_d[:])

        # persistent per-unit tiles
        qk = {}   # (u, kind, w) -> [128, WIN] f16 rope'd head window
        vh = {}   # (b, w) -> [128, 2, 4, 129] f16: per (hl, t-chunk) [V | ones]
        for u in range(B * HPC):
            for w in range(NW):
                for kind in range(2):
                    qk[(u, kind, w)] = qkp.tile(
                        [128, WIN], F16, tag=f"qk{u}_{kind}_{w}",
                        name=f"qk{u}_{kind}_{w}")
        for b in range(B):
            for w in range(NW):
                vh[(b, w)] = vhp.tile([128, 2, 4, 129], F16, tag=f"vh{b}_{w}",
                                      name=f"vh{b}_{w}")
                nc.vector.memset(vh[(b, w)][:, :, :, 128], 1.0)

        e_tiles = {}   # (b, W, hl) -> list of (exp tile, col base) per tci
        fins = {}      # (b, W) -> psum fin tile

        def stage_b(b, w):
            """Projection + rope + V for one (batch, window). Returns PE-paced
            emission items (closures)."""
            i = b * NW + w
            items = []
            for hl in range(HPC):
                u = b * HPC + hl
                for kind in range(2):
                    def proj(u=u, kind=kind, b=b, w=w, i=i):
                        ja = (kind * 4 + u % HPC * 2) * D
                        pa = psP.tile([128, WIN], F32, tag="pp",
                                      name=f"pa{b}_{w}_{u}_{kind}")
                        pb = psP.tile([128, WIN], F32, tag="pp",
                                      name=f"pb{b}_{w}_{u}_{kind}")
                        nc.tensor.matmul(pa[:], wqk_t[:, ja:ja + D], qt_w[i][:])
                        nc.tensor.matmul(pb[:], wqk_t[:, ja + D:ja + 2 * D], qt_w[i][:])
                        t1 = tmp.tile([128, WIN], F16, tag="t1",
                                      name=f"t1_{b}_{w}_{u}_{kind}")
                        nc.vector.tensor_mul(t1[:], pa[:], cs_w[w][:, 0, :])
                        t2 = tmp.tile([128, WIN], F16, tag="t2",
                                      name=f"t2_{b}_{w}_{u}_{kind}")
                        nc.vector.tensor_mul(t2[:], pb[:], cs_w[w][:, 1, :])
                        nc.gpsimd.tensor_add(qk[(u, kind, w)][:], t1[:], t2[:])
                    items.append(proj)
            for sp in range(2):
                def vproj(b=b, w=w, i=i, sp=sp):
                    psv = psP.tile([128, 2, 2, 128], F32, tag="pp",
                                   name=f"psv{b}_{w}_{sp}")
                    for k in range(2):
                        sub = 2 * sp + k
                        nc.tensor.matmul(
                            psv[:, k, :, :],
                            qt_w[i][:, sub * 128:(sub + 1) * 128], wv_t[:])
                    nc.vector.tensor_copy(
                        vh[(b, w)][:, :, 2 * sp:2 * sp + 2, 0:128].rearrange(
                            "p a b c -> p b a c"),
                        psv[:, :, :, :])
                items.append(vproj)
            return items

        def stage_s(b, W, hl):
            """scores + exp items for one (batch, query-window, head)."""
            u = b * HPC + hl
            es = []
            e_tiles[(b, W, hl)] = es
            items = []
            for tci in range(4 * W + 4):
                def score(b=b, W=W, hl=hl, u=u, tci=tci):
                    off = tci * 128 - W * WIN
                    jlo = max(0, off)
                    sc = psS.tile([128, WIN], F32, tag="ps",
                                  name=f"sc_{b}_{W}_{hl}_{tci}")
                    e_t = expp.tile([128, WIN], F16, tag="e",
                                    name=f"e_{b}_{W}_{hl}_{tci}")
                    nc.tensor.matmul(
                        sc[:, jlo:WIN],
                        qk[(u, 1, tci // 4)][:, (tci % 4) * 128:(tci % 4) * 128 + 128],
                        qk[(u, 0, W)][:, jlo:WIN])
                    nc.scalar.activation(
                        e_t[:, jlo:WIN], sc[:, jlo:WIN], AF.Exp, scale=float(SCALE))
                    if off >= 0:
                        nc.gpsimd.tensor_mul(
                            e_t[:, jlo:jlo + 128], e_t[:, jlo:jlo + 128], tril_t[:])
                    es.append((e_t, 0))
                items.append(score)
            return items

        def stage_a(b, W, hl):
            """AV + normalize + transpose + W_o items for one (b, window, head).
            AV runs as sub-PAIRS sharing one PSUM bank ([128,2,129])."""
            u = b * HPC + hl
            qs0 = W * WIN
            state = {}
            items = []

            def begin(b=b, W=W, hl=hl):
                state["es"] = e_tiles.pop((b, W, hl))
                state["oT"] = outp.tile([128, WIN], F16, tag="oT",
                                        name=f"oT{b}_{W}_{hl}")
            items.append(begin)

            for sp in range(2):
                def av_open(b=b, W=W, hl=hl, sp=sp):
                    state["avp"] = psA.tile([128, 2, 129], F32, tag="av",
                                            name=f"avp{b}_{W}_{hl}_{sp}")
                items.append(av_open)
                for k in range(2):
                    qc = 4 * W + 2 * sp + k
                    for t0 in range(0, qc + 1, 4):
                        def av_mms(b=b, W=W, hl=hl, sp=sp, k=k, qc=qc, t0=t0):
                            sub = 2 * sp + k
                            for tci in range(t0, min(t0 + 4, qc + 1)):
                                e_t, base = state["es"][tci]
                                nc.tensor.matmul(
                                    state["avp"][:, k, :],
                                    e_t[:, base + sub * 128:base + sub * 128 + 128],
                                    vh[(b, tci // 4)][:, hl, tci % 4, :],
                                    start=(tci == 0), stop=(tci == qc),
                                    skip_group_check=True)
                        items.append(av_mms)

                def norm(b=b, W=W, hl=hl, sp=sp):
                    avp = state["avp"]
                    rcp2 = tmp.tile([128, 2, 1], F32, tag="rcp",
                                    name=f"rcp{b}_{W}_{hl}_{sp}")
                    nc.vector.reciprocal(rcp2[:, :, 0], avp[:, :, 128])
                    o2 = tmp.tile([128, 2, 128], F16, tag="o_h",
                                  name=f"oh{b}_{W}_{hl}_{sp}")
                    nc.vector.tensor_mul(
                        o2[:], avp[:, :, 0:128],
                        rcp2[:].broadcast_to((128, 2, 128)))
                    tp = psT.tile([128, 2, 128], F16, tag="tp",
                                  name=f"tp{b}_{W}_{hl}_{sp}")
                    for k in range(2):
                        nc.tensor.transpose(tp[:, k, :], o2[:, k, :], id_t[:])
                    nc.vector.tensor_copy(
                        state["oT"][:, sp * 256:(sp + 1) * 256], tp[:])
                items.append(norm)

            def finish(b=b, W=W, hl=hl, qs0=qs0):
                oT = state["oT"]
                if hl == 0:
                    fins[(b, W)] = psF.tile([128, WIN], F32, tag="fin",
                                            name=f"fin{b}_{W}")
                nc.tensor.matmul(
                    fins[(b, W)][:], wo_t[:, hl * E:(hl + 1) * E], oT[:],
                    start=(hl == 0), stop=(hl == 1))
                if hl == 1:
                    fin = fins.pop((b, W))
                    fin_sb = outp.tile([128, WIN], F32, tag="fsb", bufs=2,
                                       name=f"fsb{b}_{W}")
                    nc.vector.tensor_copy(fin_sb[:], fin[:])
                    nc.sync.dma_start(
                        outp_d[b * E:(b + 1) * E, qs0:qs0 + WIN], fin_sb[:])
            items.append(finish)
            return items

        # Fine-grained merged emission. Per slot k=(b,w): the score/exp items
        # S(k) pace the kernel (ACT-bound); between consecutive score items we
        # interleave the deferred AV items A(k-1) and the NEXT slot's
        # projection items P(k+1), so the PE and DVE always have independent
        # work while ACT streams exps back-to-back.
        def merge(pace_items, fill_items, chunk=2):
            out = []
            j = 0
            n = len(pace_items)
            for i in range(0, n, chunk):
                out.extend(pace_items[i:i + chunk])
                jt = min(i + chunk, n) * len(fill_items) // n
                out.extend(fill_items[j:jt])
                j = jt
            out.extend(fill_items[j:])
            return out

        # batch 1 runs its windows in DESCENDING order so the final drain
        # (last slot's AV work) is the cheapest window (W=0). Its score blocks
        # need ALL batch-1 projections, so those are front-loaded as fill
        # during batch 0's last slot.
        slots = [(0, w) for w in range(NW)] + [(1, w) for w in reversed(range(NW))]
        fill_b = {0: [(0, 1)], 1: [(0, 2)], 2: [(0, 3)],
                  3: [(1, 0), (1, 1), (1, 2), (1, 3)]}
        for f in stage_b(*slots[0]):
            f()
        pendA = []
        for k, (b, w) in enumerate(slots):
            last = k + 1 == len(slots)
            if not last:
                pace = stage_s(b, w, 0) + stage_s(b, w, 1)
                fill = list(pendA)
                for bb, ww in fill_b.get(k, []):
                    fill += stage_b(bb, ww)
                for f in merge(pace, fill):
                    f()
                pendA = stage_a(b, w, 0) + stage_a(b, w, 1)
            else:
                for f in merge(stage_s(b, w, 0), list(pendA)):
                    f()
                for f in merge(stage_s(b, w, 1), stage_a(b, w, 0)):
                    f()
                for f in stage_a(b, w, 1):
                    f()

    nc.compile()
    return nc


def _get_nc():
    if "nc" not in _CACHE:
        _CACHE["nc"] = _build()
    return _CACHE["nc"]


def _host_inputs(q, W_q, W_k, W_v, W_o):
    """Shared (core-independent) host-side prep."""
    qT = np.ascontiguousarray(q.reshape(B * S, E).T).astype(np.float16)

    half = D // 2
    inv = (1.0 / (10000.0 ** (np.arange(half, dtype=np.float64) * 2.0 / D)))
    ang = np.arange(S, dtype=np.float64)[None, :] * inv[:, None]   # [half, S]
    cosT = np.repeat(np.cos(ang), 2, axis=0).astype(np.float32)    # [D, S]
    sinT = np.repeat(np.sin(ang), 2, axis=0).astype(np.float32)
    csT = np.ascontiguousarray(
        np.stack([cosT, sinT], axis=1)).astype(np.float16)         # [D, 2, S]
    tril = np.tril(np.ones((128, 128), dtype=np.float16)).T        # ti <= jj
    tril = np.ascontiguousarray(tril)
    ident = np.eye(128, dtype=np.float16)
    return qT, csT, tril, ident


def _swap_neg(w):
    """W' columns: w2[:, 2i] = -w[:, 2i+1], w2[:, 2i+1] = w[:, 2i]."""
    w2 = np.empty_like(w)
    w2[:, 0::2] = -w[:, 1::2]
    w2[:, 1::2] = w[:, 0::2]
    return w2


def kernel(q, W_q, W_k, W_v, W_o):
    q = np.asarray(q, dtype=np.float32)
    W_q = np.asarray(W_q, dtype=np.float32)
    W_k = np.asarray(W_k, dtype=np.float32)
    W_v = np.asarray(W_v, dtype=np.float32)
    W_o = np.asarray(W_o, dtype=np.float32)

    nc = _get_nc()
    qT, csT, tril, ident = _host_inputs(q, W_q, W_k, W_v, W_o)

    in_maps = []
    for c in range(NCORES):
        wqk = np.empty((E, 8 * D), dtype=np.float16)
        wv = np.empty((E, HPC * D), dtype=np.float16)
        wo = np.empty((D, HPC * E), dtype=np.float16)
        for hl in range(HPC):
            h = c * HPC + hl
            for kind, Wm in ((0, W_q), (1, W_k)):
                wslc = Wm[:, h * D:(h + 1) * D]
                ja = (kind * 4 + hl * 2) * D
                wqk[:, ja:ja + D] = wslc.astype(np.float16)
                wqk[:, ja + D:ja + 2 * D] = _swap_neg(wslc).astype(np.float16)
            wv[:, hl * D:(hl + 1) * D] = W_v[:, h * D:(h + 1) * D].astype(np.float16)
            wo[:, hl * E:(hl + 1) * E] = W_o[h * D:(h + 1) * D, :].astype(np.float16)
        in_maps.append({
            "qT": qT, "wqk": wqk, "wv": wv, "wo": wo,
            "csT": csT, "tril": tril, "ident": ident,
        })

    res = bass_utils.run_bass_kernel_spmd(
        nc, in_maps, core_ids=list(range(NCORES)),
        trace=bool(int(os.environ.get("KERNEL_TRACE", "0"))))
    _CACHE["last_result"] = res

    acc = np.zeros((B * E, S), dtype=np.float64)
    for r in res.results:
        acc += r["outp"].astype(np.float64)
    out = acc.reshape(B, E, S).transpose(0, 2, 1).astype(np.float32)
    return out


# revision 24
# speedup vs baseline: 1.2737x; 1.2737x over previous
"""Causal multi-head attention (RoPE) forward for Trainium2, sharded over 8 NeuronCores.

Problem (hardcoded): B=2, S=2048, E=128, H=16, D=128, inner=2048.
  out = softmax(causal(rope(q@Wq) @ rope(q@Wk).T / sqrt(D))) @ (q@Wv) @ Wo

Sharding: tensor-parallel over heads - core c owns heads {2c, 2c+1} for both
batches (4 attention units/core). Each core computes its heads' projections,
attention, and partial W_o output (row-shard); host sums the 8 partials.

v2 design notes (vs the f32r baseline, 195 us):
 - All matmul operands fp16 (1 cyc/row at any p-state/width; halves weight
   loads + SBUF). Accuracy budget: gate is 2e-2, fp16 lands ~1e-3.
 - Window-interleaved emission: AV blocks run one slot behind their score
   blocks so the PE always has independent matmuls while ACT drains exps
   (keeps the PE p-state ramped at 2.4 GHz).
 - PSUM slots are [128,1024] spanning 2 banks: score-chunk PAIRS share a
   slot so full-width exps are 1024 wide (halves ACT per-instr overhead);
   proj W/W' pairs share a slot so each rope mul is one 1024-wide DVE op.
 - o_h transposed via XBAR DMA (dma_start_transpose) instead of PE
   transpose + DVE evict.
 - W_o partials for both heads accumulate into ONE PSUM tile; evicted by
   direct PSUM->DRAM DMA if supported (else DVE copy).
 - Engine split: DVE = rope muls, V-evict, normalize, recip; Pool(gpsimd,
   SBUF-only!) = rope adds, diag tril mask; ACT = exp only.
 - Denominator via the 129th ones-column of [V | 1] (moving operand).
"""

import os
import sys
import numpy as np

for _p in ("/root/.axon_site", "/root/.axon_site/_ro/trn_rl_repo",
           "/root/.axon_site/_ro/pypackages", "/opt/trn_rl_repo"):
    if os.path.isdir(_p) and _p not in sys.path:
        sys.path.append(_p)

from collections import deque
from contextlib import ExitStack

import concourse.bacc as bacc
import concourse.mybir as mybir
import concourse.tile as tile
from concourse import bass_utils

F32 = mybir.dt.float32
F16 = mybir.dt.float16
AF = mybir.ActivationFunctionType

B, S, E = 2, 2048, 128
H, D = 16, 128
NCORES = 8
HPC = H // NCORES          # heads per core = 2
WIN = 512                  # token window
NW = S // WIN              # windows per batch = 4
SCALE = 1.0 / np.sqrt(D)

PSUM_DMA_OUT = False       # DMA cannot read PSUM (bass asserts SBUF/DRAM src)

_CACHE = {}


def _build():
    nc = bacc.Bacc("TRN2", target_bir_lowering=False, debug=False)

    qT_d = nc.dram_tensor("qT", [E, B * S], F16, kind="ExternalInput").ap()
    wqk_d = nc.dram_tensor("wqk", [E, 8 * D], F16, kind="ExternalInput").ap()
    wv_d = nc.dram_tensor("wv", [E, HPC * D], F16, kind="ExternalInput").ap()
    wo_d = nc.dram_tensor("wo", [D, HPC * E], F16, kind="ExternalInput").ap()
    cs_d = nc.dram_tensor("csT", [D, 2, S], F16, kind="ExternalInput").ap()
    tril_d = nc.dram_tensor("tril", [128, 128], F16, kind="ExternalInput").ap()
    id_d = nc.dram_tensor("ident", [128, 128], F16, kind="ExternalInput").ap()
    outp_d = nc.dram_tensor("outp", [B * E, S], F32, kind="ExternalOutput").ap()

    with tile.TileContext(nc) as tc, ExitStack() as ctx:
        const = ctx.enter_context(tc.tile_pool(name="const", bufs=1))
        qkp = ctx.enter_context(tc.tile_pool(name="qkp", bufs=1))
        vhp = ctx.enter_context(tc.tile_pool(name="vhp", bufs=1))
        tmp = ctx.enter_context(tc.tile_pool(name="tmp", bufs=3))
        expp = ctx.enter_context(tc.tile_pool(name="expp", bufs=44))
        outp = ctx.enter_context(tc.tile_pool(name="outp", bufs=3))
        psS = ctx.enter_context(tc.tile_pool(name="psS", bufs=2, space="PSUM"))
        psT = ctx.enter_context(tc.tile_pool(name="psT", bufs=1, space="PSUM"))
        psP = ctx.enter_context(tc.tile_pool(name="psP", bufs=2, space="PSUM"))
        psA = ctx.enter_context(tc.tile_pool(name="psA", bufs=2, space="PSUM"))
        psF = ctx.enter_context(tc.tile_pool(name="psF", bufs=1, space="PSUM"))

        # ---- constant loads, ordered so slot-0's deps land first ----
        wqk_t = const.tile([128, 8 * D], F16, tag="wqk")
        nc.sync.dma_start(wqk_t[:], wqk_d[:])
        qt_w = [None] * (B * NW)
        cs_w = [None] * NW
        for w in range(NW):
            t = const.tile([128, WIN], F16, tag=f"qt{w}", name=f"qt{w}")
            nc.sync.dma_start(t[:], qT_d[:, w * WIN:(w + 1) * WIN])
            qt_w[w] = t
            c = const.tile([128, 2, WIN], F16, tag=f"cs{w}", name=f"cs{w}")
            nc.sync.dma_start(c[:], cs_d[:, :, w * WIN:(w + 1) * WIN])
            cs_w[w] = c
        wv_t = const.tile([128, HPC * D], F16, tag="wv")
        nc.sync.dma_start(wv_t[:], wv_d[:])
        for i in range(NW, B * NW):
            t = const.tile([128, WIN], F16, tag=f"qt{i}", name=f"qt{i}")
            nc.sync.dma_start(t[:], qT_d[:, i * WIN:(i + 1) * WIN])
            qt_w[i] = t
        wo_t = const.tile([128, HPC * E], F16, tag="wo")
        nc.sync.dma_start(wo_t[:], wo_d[:])
        tril_t = const.tile([128, 128], F16, tag="tril")
        nc.sync.dma_start(tril_t[:], tril_d[:])
        id_t = const.tile([128, 128], F16, tag="ident")
        nc.sync.dma_start(id_t[:], id_d[:])

        # persistent per-unit tiles
        qk = {}   # (u, kind, w) -> [128, WIN] f16 rope'd head window
        vh = {}   # (b, w) -> [128, 2, 4, 129] f16: per (hl, t-chunk) [V | ones]
        for u in range(B * HPC):
            for w in range(NW):
                for kind in range(2):
                    qk[(u, kind, w)] = qkp.tile(
                        [128, WIN], F16, tag=f"qk{u}_{kind}_{w}",
                        name=f"qk{u}_{kind}_{w}")
        for b in range(B):
            for w in range(NW):
                vh[(b, w)] = vhp.tile([128, 2, 4, 129], F16, tag=f"vh{b}_{w}",
                                      name=f"vh{b}_{w}")
                nc.vector.memset(vh[(b, w)][:, :, :, 128], 1.0)

        e_tiles = {}   # (b, W, hl) -> list of (exp tile, col base) per tci
        fins = {}      # (b, W) -> psum fin tile

        def stage_b(b, w):
            """Projection + rope + V for one (batch, window). Returns PE-paced
            emission items (closures)."""
            i = b * NW + w
            items = []
            for hl in range(HPC):
                u = b * HPC + hl
                for kind in range(2):
                    def proj(u=u, kind=kind, b=b, w=w, i=i):
                        ja = (kind * 4 + u % HPC * 2) * D
                        pa = psP.tile([128, WIN], F32, tag="pp",
                                      name=f"pa{b}_{w}_{u}_{kind}")
                        pb = psP.tile([128, WIN], F32, tag="pp",
                                      name=f"pb{b}_{w}_{u}_{kind}")
                        nc.tensor.matmul(pa[:], wqk_t[:, ja:ja + D], qt_w[i][:])
                        nc.tensor.matmul(pb[:], wqk_t[:, ja + D:ja + 2 * D], qt_w[i][:])
                        t1 = tmp.tile([128, WIN], F16, tag="t1",
                                      name=f"t1_{b}_{w}_{u}_{kind}")
                        nc.vector.tensor_mul(t1[:], pa[:], cs_w[w][:, 0, :])
                        t2 = tmp.tile([128, WIN], F16, tag="t2",
                                      name=f"t2_{b}_{w}_{u}_{kind}")
                        nc.vector.tensor_mul(t2[:], pb[:], cs_w[w][:, 1, :])
                        nc.gpsimd.tensor_add(qk[(u, kind, w)][:], t1[:], t2[:])
                    items.append(proj)
            for sp in range(2):
                def vproj(b=b, w=w, i=i, sp=sp):
                    psv = psP.tile([128, 2, 2, 128], F32, tag="pp",
                                   name=f"psv{b}_{w}_{sp}")
                    for k in range(2):
                        sub = 2 * sp + k
                        nc.tensor.matmul(
                            psv[:, k, :, :],
                            qt_w[i][:, sub * 128:(sub + 1) * 128], wv_t[:])
                    nc.vector.tensor_copy(
                        vh[(b, w)][:, :, 2 * sp:2 * sp + 2, 0:128].rearrange(
                            "p a b c -> p b a c"),
                        psv[:, :, :, :])
                items.append(vproj)
            return items

        def stage_s(b, W, hl):
            """scores + exp items for one (batch, query-window, head)."""
            u = b * HPC + hl
            es = []
            e_tiles[(b, W, hl)] = es
            items = []
            for tci in range(4 * W + 4):
                def score(b=b, W=W, hl=hl, u=u, tci=tci):
                    off = tci * 128 - W * WIN
                    jlo = max(0, off)
                    sc = psS.tile([128, WIN], F32, tag="ps",
                                  name=f"sc_{b}_{W}_{hl}_{tci}")
                    e_t = expp.tile([128, WIN], F16, tag="e",
                                    name=f"e_{b}_{W}_{hl}_{tci}")
                    nc.tensor.matmul(
                        sc[:, jlo:WIN],
                        qk[(u, 1, tci // 4)][:, (tci % 4) * 128:(tci % 4) * 128 + 128],
                        qk[(u, 0, W)][:, jlo:WIN])
                    nc.scalar.activation(
                        e_t[:, jlo:WIN], sc[:, jlo:WIN], AF.Exp, scale=float(SCALE))
                    if off >= 0:
                        nc.gpsimd.tensor_mul(
                            e_t[:, jlo:jlo + 128], e_t[:, jlo:jlo + 128], tril_t[:])
                    es.append((e_t, 0))
                items.append(score)
            return items

        def stage_a(b, W, hl):
            """AV + normalize + transpose + W_o items for one (b, window, head).
            AV runs as sub-PAIRS sharing one PSUM bank ([128,2,129])."""
            u = b * HPC + hl
            qs0 = W * WIN
            state = {}
            items = []

            def begin(b=b, W=W, hl=hl):
                state["es"] = e_tiles.pop((b, W, hl))
                state["oT"] = outp.tile([128, WIN], F16, tag="oT",
                                        name=f"oT{b}_{W}_{hl}")
            items.append(begin)

            for sp in range(2):
                def av_open(b=b, W=W, hl=hl, sp=sp):
                    state["avp"] = psA.tile([128, 2, 129], F32, tag="av",
                                            name=f"avp{b}_{W}_{hl}_{sp}")
                items.append(av_open)
                for k in range(2):
                    qc = 4 * W + 2 * sp + k
                    for t0 in range(0, qc + 1, 4):
                        def av_mms(b=b, W=W, hl=hl, sp=sp, k=k, qc=qc, t0=t0):
                            sub = 2 * sp + k
                            for tci in range(t0, min(t0 + 4, qc + 1)):
                                e_t, base = state["es"][tci]
                                nc.tensor.matmul(
                                    state["avp"][:, k, :],
                                    e_t[:, base + sub * 128:base + sub * 128 + 128],
                                    vh[(b, tci // 4)][:, hl, tci % 4, :],
                                    start=(tci == 0), stop=(tci == qc),
                                    skip_group_check=True)
                        items.append(av_mms)

                def norm(b=b, W=W, hl=hl, sp=sp):
                    avp = state["avp"]
                    rcp2 = tmp.tile([128, 2, 1], F32, tag="rcp",
                                    name=f"rcp{b}_{W}_{hl}_{sp}")
                    nc.vector.reciprocal(rcp2[:, :, 0], avp[:, :, 128])
                    o2 = tmp.tile([128, 2, 128], F16, tag="o_h",
                                  name=f"oh{b}_{W}_{hl}_{sp}")
                    nc.vector.tensor_mul(
                        o2[:], avp[:, :, 0:128],
                        rcp2[:].broadcast_to((128, 2, 128)))
                    tp = psT.tile([128, 2, 128], F16, tag="tp",
                                  name=f"tp{b}_{W}_{hl}_{sp}")
                    for k in range(2):
                        nc.tensor.transpose(tp[:, k, :], o2[:, k, :], id_t[:])
                    nc.vector.tensor_copy(
                        state["oT"][:, sp * 256:(sp + 1) * 256], tp[:])
                items.append(norm)

            def finish(b=b, W=W, hl=hl, qs0=qs0):
                oT = state["oT"]
                if hl == 0:
                    fins[(b, W)] = psF.tile([128, WIN], F32, tag="fin",
                                            name=f"fin{b}_{W}")
                nc.tensor.matmul(
                    fins[(b, W)][:], wo_t[:, hl * E:(hl + 1) * E], oT[:],
                    start=(hl == 0), stop=(hl == 1))
                if hl == 1:
                    fin = fins.pop((b, W))
                    fin_sb = outp.tile([128, WIN], F32, tag="fsb", bufs=2,
                                       name=f"fsb{b}_{W}")
                    nc.vector.tensor_copy(fin_sb[:], fin[:])
                    nc.sync.dma_start(
                        outp_d[b * E:(b + 1) * E, qs0:qs0 + WIN], fin_sb[:])
            items.append(finish)
            return items

        # Fine-grained merged emission. Per slot k=(b,w): the score/exp items
        # S(k) pace the kernel (ACT-bound); between consecutive score items we
        # interleave the deferred AV items A(k-1) and the NEXT slot's
        # projection items P(k+1), so the PE and DVE always have independent
        # work while ACT streams exps back-to-back.
        def merge(pace_items, fill_items, chunk=2):
            out = []
            j = 0
            n = len(pace_items)
            for i in range(0, n, chunk):
                out.extend(pace_items[i:i + chunk])
                jt = min(i + chunk, n) * len(fill_items) // n
                out.extend(fill_items[j:jt])
                j = jt
            out.extend(fill_items[j:])
            return out

        slots = [(b, w) for b in range(B) for w in range(NW)]
        for f in stage_b(*slots[0]):
            f()
        pendA = []
        for k, (b, w) in enumerate(slots):
            last = k + 1 == len(slots)
            if not last:
                pace = stage_s(b, w, 0) + stage_s(b, w, 1)
                fill = list(pendA) + stage_b(*slots[k + 1])
                for f in merge(pace, fill):
                    f()
                pendA = stage_a(b, w, 0) + stage_a(b, w, 1)
            else:
                for f in merge(stage_s(b, w, 0), list(pendA)):
                    f()
                for f in merge(stage_s(b, w, 1), stage_a(b, w, 0)):
                    f()
                for f in stage_a(b, w, 1):
                    f()

    nc.compile()
    return nc


def _get_nc():
    if "nc" not in _CACHE:
        _CACHE["nc"] = _build()
    return _CACHE["nc"]


def _host_inputs(q, W_q, W_k, W_v, W_o):
    """Shared (core-independent) host-side prep."""
    qT = np.ascontiguousarray(q.reshape(B * S, E).T).astype(np.float16)

    half = D // 2
    inv = (1.0 / (10000.0 ** (np.arange(half, dtype=np.float64) * 2.0 / D)))
    ang = np.arange(S, dtype=np.float64)[None, :] * inv[:, None]   # [half, S]
    cosT = np.repeat(np.cos(ang), 2, axis=0).astype(np.float32)    # [D, S]
    sinT = np.repeat(np.sin(ang), 2, axis=0).astype(np.float32)
    csT = np.ascontiguousarray(
        np.stack([cosT, sinT], axis=1)).astype(np.float16)         # [D, 2, S]
    tril = np.tril(np.ones((128, 128), dtype=np.float16)).T        # ti <= jj
    tril = np.ascontiguousarray(tril)
    ident = np.eye(128, dtype=np.float16)
    return qT, csT, tril, ident


def _swap_neg(w):
    """W' columns: w2[:, 2i] = -w[:, 2i+1], w2[:, 2i+1] = w[:, 2i]."""
    w2 = np.empty_like(w)
    w2[:, 0::2] = -w[:, 1::2]
    w2[:, 1::2] = w[:, 0::2]
    return w2


def kernel(q, W_q, W_k, W_v, W_o):
    q = np.asarray(q, dtype=np.float32)
    W_q = np.asarray(W_q, dtype=np.float32)
    W_k = np.asarray(W_k, dtype=np.float32)
    W_v = np.asarray(W_v, dtype=np.float32)
    W_o = np.asarray(W_o, dtype=np.float32)

    nc = _get_nc()
    qT, csT, tril, ident = _host_inputs(q, W_q, W_k, W_v, W_o)

    in_maps = []
    for c in range(NCORES):
        wqk = np.empty((E, 8 * D), dtype=np.float16)
        wv = np.empty((E, HPC * D), dtype=np.float16)
        wo = np.empty((D, HPC * E), dtype=np.float16)
        for hl in range(HPC):
            h = c * HPC + hl
            for kind, Wm in ((0, W_q), (1, W_k)):
                wslc = Wm[:, h * D:(h + 1) * D]
                ja = (kind * 4 + hl * 2) * D
                wqk[:, ja:ja + D] = wslc.astype(np.float16)
                wqk[:, ja + D:ja + 2 * D] = _swap_neg(wslc).astype(np.float16)
            wv[:, hl * D:(hl + 1) * D] = W_v[:, h * D:(h + 1) * D].astype(np.float16)
            wo[:, hl * E:(hl + 1) * E] = W_o[h * D:(h + 1) * D, :].astype(np.float16)
        in_maps.append({
            "qT": qT, "wqk": wqk, "wv": wv, "wo": wo,
            "csT": csT, "tril": tril, "ident": ident,
        })

    res = bass_utils.run_bass_kernel_spmd(
        nc, in_maps, core_ids=list(range(NCORES)),
        trace=bool(int(os.environ.get("KERNEL_TRACE", "0"))))
    _CACHE["last_result"] = res

    acc = np.zeros((B * E, S), dtype=np.float64)
    for r in res.results:
        acc += r["outp"].astype(np.float64)
    out = acc.reshape(B, E, S).transpose(0, 2, 1).astype(np.float32)
    return out
